# revision 1
# baseline (speedup 1.0000x reference)
"""CRF negative-log-likelihood loss kernel for Trainium2 (8 NeuronCores).

Strategy
--------
Data-parallel over the batch: 32 sequences -> 4 per core. Each core evaluates
the log-partition function in *linear* space with a bidirectional split that
halves the serial chain: the forward recurrence

    alpha_t = (M @ alpha_{t-1}) * e_t,      M = exp(T), e_t = exp(feat_t)

runs from t=0 up to t=255 while the backward recurrence

    beta_t  = M^T-contraction of (e_{t+1} * beta_{t+1}),   beta_511 = 1

runs from t=511 down to t=255; both are one 64x64 TensorEngine matmul plus
one elementwise VectorEngine multiply per step, and the two chains interleave
on the engines so the wall time is one chain's ~256-step latency. They meet
with  Z = sum_i alpha_255[i] * beta_255[i].

The matmul weights are augmented to [W | ONES] (bf16, single PE pass), so
every step's psum also delivers sum_i(state) broadcast across rows 64-127.
Every K=8 steps that sum renormalizes the chain: the reciprocal is folded
into a later step's emission operand (scale-invariance), so normalization
never touches the serial critical path. Each log-scale is evaluated as
Ln(s * 2^-48) (ScalarE Ln saturates at 2^64) and the 48*ln2 is added back at
the end.

The gold-path score (emissions at tags plus transitions) is computed with
one-hot matmuls: per sequence, G = [F | OH_next]^T @ OH_prev has feats^T@OH
in rows 0-63 (diagonal = emission score) and the transition-pair count matrix
in rows 64-127 (Frobenius product with T = transition score); one multiply
with [I; T], a row reduce, and a ones-matmul collapse it to scalars. All off
the critical path.

Host-side work is limited to input relayout: per-core slicing, one-hot
encoding of the integer tags (with a zero guard row), transposing T, and
concatenating eye(64) with T.
"""

import math

import numpy as np
from contextlib import ExitStack

B, T_LEN, L = 32, 512, 64
N_CORES = 8
BPC = B // N_CORES  # sequences per core
T_MID = 255         # chains meet here
K_NORM = 10         # renormalize every K steps
LN_SCALE = 2.0 ** -48

_compiled = None  # compiled program cache so repeated kernel() calls reuse it


def _build_program():
    import concourse.bacc as bacc
    import concourse.tile as tile
    import concourse.mybir as mybir
    from concourse.alu_op_type import AluOpType

    f32 = mybir.dt.float32
    bf16 = mybir.dt.bfloat16
    Af = mybir.ActivationFunctionType

    nc = bacc.Bacc("TRN2", target_bir_lowering=False, debug=False,
                   num_devices=N_CORES)

    # feats arrives t-major: row t*BPC+b holds feats[b, t, :]
    feats_d = nc.dram_tensor("feats", [BPC * T_LEN, L], f32,
                             kind="ExternalInput").ap()
    oh_d = nc.dram_tensor("oh", [BPC * (T_LEN + 1), L], f32,
                          kind="ExternalInput").ap()
    tt_d = nc.dram_tensor("tt", [L, L], f32, kind="ExternalInput").ap()
    mask_d = nc.dram_tensor("mask", [2 * L, L], f32, kind="ExternalInput").ap()
    out_d = nc.dram_tensor("out", [1, BPC], f32, kind="ExternalOutput").ap()

    with tile.TileContext(nc) as tc, ExitStack() as ctx:
        consts = ctx.enter_context(tc.tile_pool(name="consts", bufs=1))
        loadp = ctx.enter_context(tc.tile_pool(name="load", bufs=1))
        goldp = ctx.enter_context(tc.tile_pool(name="gold", bufs=16))
        alphap = ctx.enter_context(tc.tile_pool(name="alpha", bufs=4))
        vtmp = ctx.enter_context(tc.tile_pool(name="vtmp", bufs=6))
        qf = ctx.enter_context(tc.tile_pool(name="qfpsum", bufs=3, space="PSUM"))
        qb = ctx.enter_context(tc.tile_pool(name="qbpsum", bufs=3, space="PSUM"))
        tpp = ctx.enter_context(tc.tile_pool(name="tpsum", bufs=2, space="PSUM"))

        # ---- constants ----
        ones128 = consts.tile([128, 1], f32)
        nc.gpsimd.memset(ones128[:], 1.0)
        mask_sb = consts.tile([128, L], f32)
        nc.sync.dma_start(out=mask_sb[:], in_=mask_d)
        ttile = consts.tile([L, L], f32)          # T^T
        nc.sync.dma_start(out=ttile[:], in_=tt_d)
        tstr = consts.tile([L, L], f32)           # T (straight)
        nc.sync.dma_start(out=tstr[:], in_=mask_d[L:2 * L, :])
        # W3  = [exp(T)^T | ONES]  (forward);  W3b = [exp(T) | ONES] (backward)
        # Matmul against either gives the new state in psum rows 0-63 and the
        # input-state column sums broadcast across rows 64-127.
        W3 = consts.tile([L, 2 * L], bf16)
        nc.scalar.activation(W3[:, 0:L], ttile[:], Af.Exp)
        nc.gpsimd.memset(W3[:, L:2 * L], 1.0)
        W3b = consts.tile([L, 2 * L], bf16)
        nc.scalar.activation(W3b[:, 0:L], tstr[:], Af.Exp)
        nc.gpsimd.memset(W3b[:, L:2 * L], 1.0)

        # ---- e_feats in t-major chunks: efc[k][j, (t%32)*4 + b] (bf16) ----
        # Per chunk: contiguous-ish DMA of 128 t-major rows, Exp -> bf16 into
        # the left half of a [128,128] staging tile, then an xbar
        # DMA-transpose (2-byte dtype, free%128) whose partitions 0-63 are the
        # transposed chunk. No TensorEngine involvement, so the recurrence
        # matmuls never hit a PE tiling-mode switch. Chunks are emitted in the
        # order the two chains consume them (0, 15, 1, 14, ...).
        # Chunks are packed in pairs (w, 15-w) so one [128,128] xbar
        # transpose yields forward chunk w on partitions 0-63 and backward
        # chunk 15-w on partitions 64-127 (moved down by a small SBUF copy).
        # Window 0 already provides both chains' first chunks, so the
        # recurrence starts after one transpose. Wide Exps (4x [128,256])
        # replace 16 small ones; all copies precede all transposes to keep
        # DMA copy<->transpose mode transitions rare (Tile serializes them).
        packed = []
        for w in range(8):
            packed += [w, 15 - w]
        fcs, stgs = [None] * 4, [None] * 4
        eks, mvs = [None] * 8, [None] * 8
        last_mv = None

        def load_group(g):
            fcg = loadp.tile([128, 4 * L], f32, tag=f"fc{g}")
            for j in range(4):
                ck = packed[4 * g + j]
                nc.sync.dma_start(out=fcg[:, j * L:(j + 1) * L],
                                  in_=feats_d[ck * 128:(ck + 1) * 128, :])
            fcs[g] = fcg
            stg = loadp.tile([128, 4 * L], bf16, tag=f"stg{g}")
            nc.scalar.activation(stg[:], fcg[:], Af.Exp)
            stgs[g] = stg

        def transpose_window(w):
            nonlocal last_mv
            ek = consts.tile([128, 128], bf16, tag=f"ef{w}")
            nc.sync.dma_start(
                out=ek[:],
                in_=stgs[w // 2][:, (w % 2) * 128:(w % 2) * 128 + 128],
                transpose=True)
            eks[w] = ek
            mv = consts.tile([L, 128], bf16, tag=f"mv{w}")
            last_mv = nc.sync.dma_start(out=mv[:], in_=ek[L:128, :])
            mvs[w] = mv

        # group 0 end-to-end first: both chains' first chunks (0 and 15) are
        # ready after one transpose, so the recurrence starts ~10us earlier.
        load_group(0)
        transpose_window(0)
        transpose_window(1)
        for g in (1, 2, 3):
            load_group(g)
        for w in range(2, 8):
            transpose_window(w)

        def ef_col(t):  # [64, 4] AP of exp(feats[:, t, :]) for the 4 seqs
            k, col = t // 32, 4 * (t % 32)
            if k <= 7:
                return eks[k][0:L, col:col + 4]
            return mvs[15 - k][:, col:col + 4]

        # ---- bidirectional recurrence ----
        fwd_ev_tmp = set(range(K_NORM, T_MID - 2, K_NORM)) | {T_MID}
        bwd_ev_tmp = (set(range(T_LEN - 1 - K_NORM, T_MID + 3, -K_NORM))
                      | {T_MID + 1})
        n_events = len(fwd_ev_tmp) + len(bwd_ev_tmp) + 1
        lnS = consts.tile([1, 4 * n_events], f32)
        ev = 0

        def emit_ln(ps_row):  # ps_row: [1, BPC] psum AP holding s
            nonlocal ev
            nc.scalar.activation(lnS[:, 4 * ev:4 * ev + 4], ps_row,
                                 Af.Ln, scale=LN_SCALE)
            ev += 1

        alpha = alphap.tile([L, BPC], bf16, tag="alpha")
        nc.vector.tensor_copy(alpha[:], ef_col(0))
        v = alphap.tile([L, BPC], bf16, tag="v")
        nc.vector.tensor_copy(v[:], ef_col(T_LEN - 1))

        es_f = {}   # fwd step -> prescaled emission operand
        es_b = {}   # bwd step -> prescaled emission operand
        fwd_events = fwd_ev_tmp
        bwd_events = bwd_ev_tmp

        for s in range(T_MID):
            tf = 1 + s          # forward step index
            tb = T_LEN - 2 - s  # backward step index (mul at tb)

            # forward: q = W3^T @ alpha ; alpha = q[0:64] * e
            q = qf.tile([2 * L, BPC], f32, tag="q")
            nc.tensor.matmul(q[:], lhsT=W3[:], rhs=alpha[:],
                             start=True, stop=True)
            eop = es_f.pop(tf, None)
            if eop is None:
                eop = ef_col(tf)
            alpha_new = alphap.tile([L, BPC], bf16, tag="alpha")
            nc.vector.tensor_mul(alpha_new[:], q[0:L, :], eop)
            alpha = alpha_new
            if tf + 2 in fwd_events:  # 1/s(alpha_{tf-1}) lands at step tf+2
                rvf = vtmp.tile([L, BPC], f32, tag="rvf")
                nc.vector.reciprocal(rvf[:], q[L:2 * L, :])
                esf = vtmp.tile([L, BPC], f32, tag="esf")
                nc.gpsimd.tensor_mul(esf[:], ef_col(tf + 2), rvf[:])
                emit_ln(q[L:L + 1, :])
                es_f[tf + 2] = esf

            # backward: p = W3b^T @ v_{tb+1} ; v_tb = p[0:64] * e_tb
            p = qb.tile([2 * L, BPC], f32, tag="p")
            nc.tensor.matmul(p[:], lhsT=W3b[:], rhs=v[:],
                             start=True, stop=True)
            eop = es_b.pop(tb, None)
            if eop is None:
                eop = ef_col(tb)
            v_new = alphap.tile([L, BPC], bf16, tag="v")
            nc.vector.tensor_mul(v_new[:], p[0:L, :], eop)
            v = v_new
            if tb - 2 in bwd_events:
                rvb = vtmp.tile([L, BPC], f32, tag="rvb")
                nc.vector.reciprocal(rvb[:], p[L:2 * L, :])
                esb = vtmp.tile([L, BPC], f32, tag="esb")
                nc.gpsimd.tensor_mul(esb[:], ef_col(tb - 2), rvb[:])
                emit_ln(p[L:L + 1, :])
                es_b[tb - 2] = esb

        assert not es_f and not es_b, (sorted(es_f), sorted(es_b))
        # last backward contraction down to T_MID (no emission at T_MID here:
        # alpha_255 already carries e_255)
        p = qb.tile([2 * L, BPC], f32, tag="p")
        nc.tensor.matmul(p[:], lhsT=W3b[:], rhs=v[:], start=True, stop=True)

        # combine: Z_core = sum_i alpha_255[i] * beta_255[i]
        g = alphap.tile([L, BPC], bf16, tag="alpha")
        nc.vector.tensor_mul(g[:], p[0:L, :], alpha[:])
        qz = qf.tile([2 * L, BPC], f32, tag="q")
        qz_inst = nc.tensor.matmul(qz[:], lhsT=W3[:], rhs=g[:],
                                   start=True, stop=True)
        emit_ln(qz[L:L + 1, :])
        assert ev == n_events, ev

        fwd = vtmp.tile([1, BPC], f32, tag="fwd")
        nc.vector.tensor_reduce(
            fwd[:], lnS[:].rearrange("p (n b) -> p b n", b=BPC),
            axis=mybir.AxisListType.X, op=AluOpType.add)
        # add back the n_events * 48*ln2 removed by the Ln pre-scale
        lnoff = consts.tile([1, BPC], f32)
        nc.gpsimd.memset(lnoff[:], float(n_events * 48.0 * math.log(2.0)))
        fwd2 = vtmp.tile([1, BPC], f32, tag="fwd2")
        nc.vector.tensor_add(fwd2[:], fwd[:], lnoff[:])

        # ---- gold score via one-hot matmuls, forced after the loop ----
        # feats_d is t-major, so the F operand reads per-sequence strided
        # rows; oh stays (b t)-major with contiguous reads. Every gold matmul
        # gets an explicit dependency on the loop's final matmul: its
        # (128,128) PE tiling mode would otherwise interleave with the
        # (64,128) recurrence matmuls and each switch drains the PE.
        from concourse.tile_rust import add_dep_helper
        feats_bmaj = feats_d.rearrange("(t b) l -> b t l", b=BPC)
        Vt = consts.tile([128, BPC], f32)
        for b in range(BPC):
            gps = tpp.tile([128, L], f32, tag="tp")
            for c in range(4):
                o0 = b * (T_LEN + 1) + c * 128
                cat = goldp.tile([128, 128], f32, tag="cat")
                d1 = nc.sync.dma_start(
                    out=cat[:, 0:L],
                    in_=feats_bmaj[b, c * 128:(c + 1) * 128, :])
                d2 = nc.sync.dma_start(out=cat[:, L:2 * L],
                                       in_=oh_d[o0 + 1:o0 + 129, :])
                ohp = goldp.tile([128, L], f32, tag="ohp")
                d3 = nc.sync.dma_start(out=ohp[:], in_=oh_d[o0:o0 + 128, :])
                for dd in (d1, d2, d3):
                    add_dep_helper(dd.ins, last_mv.ins, sync=True,
                                   reason="gold copies after xbar transposes")
                gi = nc.tensor.matmul(gps[:], lhsT=cat[:], rhs=ohp[:],
                                      start=(c == 0), stop=(c == 3))
                add_dep_helper(gi.ins, qz_inst.ins, sync=True,
                               reason="gold matmuls after recurrence")
            gsc = vtmp.tile([128, L], f32, tag="gsc")
            nc.vector.tensor_mul(gsc[:], gps[:], mask_sb[:])
            nc.vector.tensor_reduce(Vt[:, b:b + 1], gsc[:],
                                    axis=mybir.AxisListType.X,
                                    op=AluOpType.add)
        gold_ps = tpp.tile([128, L], f32, tag="tp")
        nc.tensor.matmul(gold_ps[0:1, 0:BPC], lhsT=ones128[:, 0:1], rhs=Vt[:],
                         start=True, stop=True)

        res = vtmp.tile([1, BPC], f32, tag="res")
        nc.vector.tensor_tensor(res[:], fwd2[:], gold_ps[0:1, 0:BPC],
                                op=AluOpType.subtract)
        nc.sync.dma_start(out=out_d, in_=res[:])

    import concourse.bacc as bacc2
    orig = bacc2.Bacc.move_matmul_waits_to_ldweights
    if SKIP_LDW_WAIT_PASS:
        # Keep semaphore waits on the MATMUL itself so the (constant-weight)
        # LDWEIGHTS can issue while the previous step's DVE multiply runs.
        bacc2.Bacc.move_matmul_waits_to_ldweights = lambda self: None
    try:
        nc.compile()
    finally:
        bacc2.Bacc.move_matmul_waits_to_ldweights = orig
    return nc


SKIP_LDW_WAIT_PASS = True


def _prep_in_maps(feats, tags, T):
    feats = np.ascontiguousarray(np.asarray(feats, dtype=np.float32))
    T_np = np.ascontiguousarray(np.asarray(T, dtype=np.float32))
    tags_np = np.asarray(tags).astype(np.int64)

    oh = np.zeros((B, T_LEN + 1, L), dtype=np.float32)
    oh[np.arange(B)[:, None], np.arange(T_LEN)[None, :], tags_np] = 1.0
    mask_const = np.concatenate([np.eye(L, dtype=np.float32), T_np], axis=0)
    tt = np.ascontiguousarray(T_np.T)

    in_maps = []
    for c in range(N_CORES):
        sl = slice(c * BPC, (c + 1) * BPC)
        in_maps.append({
            "feats": np.ascontiguousarray(
                feats[sl].transpose(1, 0, 2).reshape(T_LEN * BPC, L)),
            "oh": np.ascontiguousarray(
                oh[sl].reshape(BPC * (T_LEN + 1), L)),
            "tt": tt,
            "mask": mask_const,
        })
    return in_maps


def kernel(feats, tags, T):
    global _compiled
    from concourse.bass_utils import run_bass_kernel_spmd

    if _compiled is None:
        _compiled = _build_program()
    nc = _compiled

    in_maps = _prep_in_maps(feats, tags, T)
    res = run_bass_kernel_spmd(nc, in_maps, list(range(N_CORES)))
    out = np.concatenate(
        [res.results[c]["out"].reshape(BPC) for c in range(N_CORES)])
    return out.astype(np.float32)



# revision 6
# speedup vs baseline: 1.2492x; 1.2492x over previous
"""CRF negative-log-likelihood loss kernel for Trainium2 (8 NeuronCores).

Strategy
--------
Data-parallel over the batch: 32 sequences -> 4 per core. The log-partition
function is evaluated in linear space with a 4-way split of the time axis
that cuts the serial chain to 85 rounds:

  vec-fwd : alpha_t = e_t * (M @ alpha_{t-1}),  t = 1..170   (M = exp(T))
  matA    : X <- D_s (M^T X), s = 254..171, X0 = diag(e_255)
            => alpha_255 = X_A^T (M alpha_170)   [64x64 transfer matrix]
  matB    : X <- D_s (M^T X), s = 339..256, X0 = diag(e_340)
            => alpha_340 = X_B^T (M alpha_255)
  vec-bwd : v_t = e_t * (M^T v_{t+1}),  t = 510..341, v_511 = e_511
  combine : Z = (M alpha_340) . v_341

Both vector chains live stacked in one [128,4] state: one matmul against a
constant block-diagonal weight blkdiag(exp(T)^T, exp(T)) advances fwd+bwd
together; one DVE multiply applies both emissions. The two matrix chains
live stacked in one [128, 4*64] state with weight blkdiag(exp(T), exp(T));
their per-step row-scale by e_s reads the emission tile through a stride-0
broadcast AP, split across DVE and GpSimd (both can read PSUM) so the DVE
stays under the round budget. Each round advances the vector chains TWO
steps and the matrix chains ONE step (the matrix step's [128,256] psum->sbuf
move is slower, 171/85 balances the two serial cadences).

Emissions are prescaled: e_t = exp(feat_t - C) with C = ln(64)+1 (the mean
per-step growth), which keeps every chain's magnitude within e^+-8 over the
whole sequence - NO renormalization events at all. The final
logZ = Ln(sum g) + 512*C.

The gold-path score (emissions at tags plus transitions) uses one-hot
matmuls identical to the loop's (128,128) PE tiling, placed after the loop.

Host-side work is limited to input relayout: window packing of feats (each
128x128 window pairs two 32-step chunks so one DMA-transpose yields both
chains' emission columns), one-hot tag encoding, and tiny init diagonals.
"""

import math

import numpy as np
from contextlib import ExitStack

B, T_LEN, L = 32, 512, 64
N_CORES = 8
BPC = B // N_CORES   # sequences per core
V = 171              # vector-chain steps per direction (e_0..e_170 fwd)
MS = 85              # matrix-chain steps per direction
C_PRE = math.log(64.0) + 1.0
N_ROUNDS = MS        # 85 rounds; 2 vec steps + 1 mat step per round
N_VWIN = 6           # vec windows (ceil(171/32))
N_MWIN = 3           # mat windows (ceil(85/32))

_compiled = None


def _build_program():
    import concourse.bacc as bacc
    import concourse.tile as tile
    import concourse.mybir as mybir
    from concourse.alu_op_type import AluOpType

    f32 = mybir.dt.float32
    bf16 = mybir.dt.bfloat16
    Af = mybir.ActivationFunctionType

    nc = bacc.Bacc("TRN2", target_bir_lowering=False, debug=False,
                   num_devices=N_CORES)

    NW = N_VWIN + N_MWIN
    wins_d = nc.dram_tensor("wins", [NW * 128, 128], f32,
                            kind="ExternalInput").ap()
    xinit_d = nc.dram_tensor("xinit", [128, 4 * L], f32,
                             kind="ExternalInput").ap()
    consts_d = nc.dram_tensor("consts", [5 * L, L], f32,
                              kind="ExternalInput").ap()
    fgold_d = nc.dram_tensor("fgold", [BPC * T_LEN, L], f32,
                             kind="ExternalInput").ap()
    oh_d = nc.dram_tensor("oh", [BPC * (T_LEN + 1), L], f32,
                          kind="ExternalInput").ap()
    out_d = nc.dram_tensor("out", [1, BPC], f32, kind="ExternalOutput").ap()

    with tile.TileContext(nc) as tc, ExitStack() as ctx:
        consts = ctx.enter_context(tc.tile_pool(name="consts", bufs=1))
        loadp = ctx.enter_context(tc.tile_pool(name="load", bufs=3))
        goldp = ctx.enter_context(tc.tile_pool(name="gold", bufs=32))
        alphap = ctx.enter_context(tc.tile_pool(name="alpha", bufs=6))
        xp = ctx.enter_context(tc.tile_pool(name="xstate", bufs=3))
        vtmp = ctx.enter_context(tc.tile_pool(name="vtmp", bufs=6))
        qv = ctx.enter_context(tc.tile_pool(name="qvpsum", bufs=3,
                                            space="PSUM"))
        qm = ctx.enter_context(tc.tile_pool(name="qmpsum", bufs=3,
                                            space="PSUM"))
        tpp = ctx.enter_context(tc.tile_pool(name="tpsum", bufs=2,
                                             space="PSUM"))

        # ---- constants ----
        ones128 = consts.tile([128, 1], f32)
        nc.gpsimd.memset(ones128[:], 1.0)
        ones128b = consts.tile([128, 1], bf16)
        nc.gpsimd.memset(ones128b[:], 1.0)
        mask_sb = consts.tile([128, L], f32)     # [eye(64); T]
        nc.sync.dma_start(out=mask_sb[:], in_=consts_d[0:2 * L, :])
        tt0 = consts.tile([L, L], f32)           # T^T at partitions 0:64
        nc.sync.dma_start(out=tt0[:], in_=consts_d[2 * L:3 * L, :])
        ts0 = consts.tile([L, L], f32)           # T  at partitions 0:64
        nc.sync.dma_start(out=ts0[:], in_=consts_d[3 * L:4 * L, :])
        ttT = consts.tile([128, L], f32)         # T^T at partitions 64:128
        nc.sync.dma_start(out=ttT[L:128, :], in_=consts_d[4 * L:5 * L, :])
        xi_f = consts.tile([128, 4 * L], f32)
        nc.sync.dma_start(out=xi_f[:], in_=xinit_d)

        # W_vec = blkdiag(exp(T)^T, exp(T)); W_mat = blkdiag(exp(T), exp(T))
        Wv = consts.tile([128, 128], bf16)
        nc.gpsimd.memset(Wv[:], 0.0)
        nc.scalar.activation(Wv[0:L, 0:L], tt0[:], Af.Exp)
        nc.scalar.activation(Wv[L:128, L:128], mask_sb[L:128, :], Af.Exp)
        Wm = consts.tile([128, 128], bf16)
        nc.gpsimd.memset(Wm[:], 0.0)
        nc.scalar.activation(Wm[0:L, 0:L], ts0[:], Af.Exp)
        nc.scalar.activation(Wm[L:128, L:128], mask_sb[L:128, :], Af.Exp)
        # exp(T)^T at partitions 64:128 (tail weight for y3 = M a3)
        Wy3 = consts.tile([128, L], bf16)
        nc.scalar.activation(Wy3[L:128, :], ttT[L:128, :], Af.Exp)

        # ---- emission windows: DMA -> Exp(x - C) bf16 -> DMA-transpose ----
        # Window tile ek[k]: [128 part = (chainTop L | chainBot L),
        #                     128 free = 4*pos + b].
        negC = consts.tile([128, 1], f32)
        nc.gpsimd.memset(negC[:], -C_PRE)
        eks = [None] * NW

        def load_window(k):
            fcg = loadp.tile([128, 128], f32, tag=f"fc{k}")
            nc.sync.dma_start(out=fcg[:],
                              in_=wins_d[k * 128:(k + 1) * 128, :])
            stg = loadp.tile([128, 128], bf16, tag=f"stg{k}")
            nc.scalar.activation(stg[:], fcg[:], Af.Exp, bias=negC[:])
            return stg

        def transpose_window(k, stg):
            ek = consts.tile([128, 128], bf16, tag=f"ef{k}")
            nc.sync.dma_start(out=ek[:], in_=stg[:], transpose=True)
            eks[k] = ek

        # first vec + first mat window end-to-end, then the rest
        stg0 = load_window(0)
        stgm = load_window(N_VWIN)
        transpose_window(0, stg0)
        transpose_window(N_VWIN, stgm)
        rest = [k for k in range(NW) if k not in (0, N_VWIN)]
        stgs = {k: load_window(k) for k in rest}
        for k in rest:
            transpose_window(k, stgs[k])

        def e_vec(idx):   # [128,4] emissions for vec round-step idx
            w, p = idx // 32, idx % 32
            return eks[w][:, 4 * p:4 * p + 4]

        def e_mat(idx):   # [128,4] emissions for mat step idx
            w, p = idx // 32, idx % 32
            return eks[N_VWIN + w][:, 4 * p:4 * p + 4]

        # ---- init states ----
        S = alphap.tile([128, BPC], bf16, tag="S")
        nc.vector.tensor_copy(S[:], e_vec(0))      # [alpha_0 ; v_511]
        X = xp.tile([128, 4 * L], bf16, tag="X")
        nc.vector.tensor_copy(X[:], xi_f[:])       # [diag e_255 ; diag e_340]

        # ---- main loop: 2 vec steps + 1 mat step per round ----
        def vec_step(idx):
            nonlocal S
            q = qv.tile([128, BPC], f32, tag="q")
            nc.tensor.matmul(q[:], lhsT=Wv[:], rhs=S[:],
                             start=True, stop=True)
            S_new = alphap.tile([128, BPC], bf16, tag="S")
            nc.vector.tensor_mul(S_new[:], q[:], e_vec(idx))
            S = S_new

        for r in range(N_ROUNDS):
            vec_step(1 + 2 * r)
            if r < MS - 1:
                p = qm.tile([128, 4 * L], f32, tag="p")
                nc.tensor.matmul(p[:], lhsT=Wm[:], rhs=X[:],
                                 start=True, stop=True)
                em = e_mat(1 + r)
                X_new = xp.tile([128, 4 * L], bf16, tag="X")
                # row-scale by e: free dim (b, i) with e broadcast over i
                pb = p[:].rearrange("a (b i) -> a b i", b=4)
                xb = X_new[:].rearrange("a (b i) -> a b i", b=4)
                nc.vector.tensor_mul(
                    xb[:], pb[:],
                    em[:].unsqueeze(2).broadcast_to((128, 4, L)))
                X = X_new
            vec_step(2 + 2 * r)

        # ---- tail: stitch the four chains together ----
        # y1 = M alpha_170 (rows 0:64 of a W_vec matmul)
        qy1 = qv.tile([128, BPC], f32, tag="q")
        nc.tensor.matmul(qy1[:], lhsT=Wv[:], rhs=S[:], start=True, stop=True)
        y1 = vtmp.tile([L, BPC], bf16, tag="y1")
        nc.vector.tensor_copy(y1[:], qy1[0:L, :])
        # a2 = X_A^T y1  (per-sequence 64x64 blocks, rows 0:64 of X)
        qa2 = qv.tile([128, BPC], f32, tag="q")
        for b in range(BPC):
            nc.tensor.matmul(qa2[0:L, b:b + 1],
                             lhsT=X[0:L, b * L:(b + 1) * L],
                             rhs=y1[:, b:b + 1], start=True, stop=True)
        a2 = vtmp.tile([L, BPC], bf16, tag="a2")
        nc.vector.tensor_copy(a2[:], qa2[0:L, :])
        # y2 = M a2 -> psum partitions 64:128 (to meet X_B's partition base)
        qy2 = qv.tile([128, BPC], f32, tag="q")
        nc.tensor.matmul(qy2[L:128, :], lhsT=Wv[0:L, 0:L], rhs=a2[:],
                         start=True, stop=True, tile_position=(0, 64))
        y2 = vtmp.tile([128, BPC], bf16, tag="y2")
        nc.vector.tensor_copy(y2[L:128, :], qy2[L:128, :])
        # a3 = X_B^T y2  (rows 64:128 of X)
        qa3 = qv.tile([128, BPC], f32, tag="q")
        for b in range(BPC):
            nc.tensor.matmul(qa3[L:128, b:b + 1],
                             lhsT=X[L:128, b * L:(b + 1) * L],
                             rhs=y2[L:128, b:b + 1], start=True, stop=True,
                             tile_position=(64, 64))
        a3 = vtmp.tile([128, BPC], bf16, tag="a3")
        nc.vector.tensor_copy(a3[L:128, :], qa3[L:128, :])
        # y3 = M a3; g = y3 * v_341; Z = sum(g)
        qy3 = qv.tile([128, BPC], f32, tag="q")
        nc.tensor.matmul(qy3[L:128, :], lhsT=Wy3[L:128, :], rhs=a3[L:128, :],
                         start=True, stop=True, tile_position=(64, 64))
        g = vtmp.tile([128, BPC], bf16, tag="g")
        nc.vector.tensor_mul(g[L:128, :], qy3[L:128, :], S[L:128, :])
        qz = qv.tile([128, BPC], f32, tag="q")
        nc.tensor.matmul(qz[0:1, :], lhsT=ones128b[L:128, 0:1],
                         rhs=g[L:128, :], start=True, stop=True,
                         tile_position=(64, 0))
        lnZ = vtmp.tile([1, BPC], f32, tag="lnZ")
        nc.scalar.activation(lnZ[:], qz[0:1, :], Af.Ln)
        lnoff = consts.tile([1, BPC], f32)
        nc.gpsimd.memset(lnoff[:], float(T_LEN * C_PRE))
        fwd2 = vtmp.tile([1, BPC], f32, tag="fwd2")
        nc.vector.tensor_add(fwd2[:], lnZ[:], lnoff[:])

        # ---- gold score via one-hot matmuls (same (128,128) PE tiling) ----
        Vt = consts.tile([128, BPC], f32)
        for b in range(BPC):
            gps = tpp.tile([128, L], f32, tag="tp")
            for c in range(4):
                o0 = b * (T_LEN + 1) + c * 128
                cat = goldp.tile([128, 128], f32, tag="cat")
                nc.sync.dma_start(
                    out=cat[:, 0:L],
                    in_=fgold_d[b * T_LEN + c * 128:b * T_LEN + (c + 1) * 128,
                                :])
                nc.sync.dma_start(out=cat[:, L:2 * L],
                                  in_=oh_d[o0 + 1:o0 + 129, :])
                ohp = goldp.tile([128, L], f32, tag="ohp")
                nc.sync.dma_start(out=ohp[:], in_=oh_d[o0:o0 + 128, :])
                nc.tensor.matmul(gps[:], lhsT=cat[:], rhs=ohp[:],
                                 start=(c == 0), stop=(c == 3))
            gsc = vtmp.tile([128, L], f32, tag="gsc")
            nc.vector.tensor_mul(gsc[:], gps[:], mask_sb[:])
            nc.vector.tensor_reduce(Vt[:, b:b + 1], gsc[:],
                                    axis=mybir.AxisListType.X,
                                    op=AluOpType.add)
        gold_ps = tpp.tile([128, L], f32, tag="tp")
        nc.tensor.matmul(gold_ps[0:1, 0:BPC], lhsT=ones128[:, 0:1], rhs=Vt[:],
                         start=True, stop=True)

        res = vtmp.tile([1, BPC], f32, tag="res")
        nc.vector.tensor_tensor(res[:], fwd2[:], gold_ps[0:1, 0:BPC],
                                op=AluOpType.subtract)
        nc.sync.dma_start(out=out_d, in_=res[:])

    import concourse.bacc as bacc2
    orig = bacc2.Bacc.move_matmul_waits_to_ldweights
    if SKIP_LDW_WAIT_PASS:
        # Keep semaphore waits on the MATMUL itself so the LDWEIGHTS can
        # issue while the previous round's DVE multiply runs.
        bacc2.Bacc.move_matmul_waits_to_ldweights = lambda self: None
    try:
        nc.compile()
    finally:
        bacc2.Bacc.move_matmul_waits_to_ldweights = orig
    return nc


SKIP_LDW_WAIT_PASS = True


def _prep_in_maps(feats, tags, T):
    feats = np.ascontiguousarray(np.asarray(feats, dtype=np.float32))
    T_np = np.ascontiguousarray(np.asarray(T, dtype=np.float32))
    tags_np = np.asarray(tags).astype(np.int64)

    oh = np.zeros((B, T_LEN + 1, L), dtype=np.float32)
    oh[np.arange(B)[:, None], np.arange(T_LEN)[None, :], tags_np] = 1.0
    consts = np.concatenate(
        [np.eye(L, dtype=np.float32), T_np, T_np.T, T_np, T_np.T], axis=0)
    consts = np.ascontiguousarray(consts)

    pos = np.arange(32)
    NW = N_VWIN + N_MWIN
    in_maps = []
    for c in range(N_CORES):
        fc = feats[c * BPC:(c + 1) * BPC]          # [4, 512, 64]
        wins = np.zeros((NW, 128, 128), dtype=np.float32)
        for w in range(N_VWIN):
            t_top = np.minimum(32 * w + pos, T_LEN - 1)
            t_bot = T_LEN - 1 - 32 * w - pos
            # [4, 32, 64] -> [32, 4, 64] -> [128, 64]
            wins[w, :, 0:L] = fc[:, t_top].transpose(1, 0, 2).reshape(128, L)
            wins[w, :, L:2 * L] = (
                fc[:, t_bot].transpose(1, 0, 2).reshape(128, L))
        for w in range(N_MWIN):
            t_a = V + MS - 1 - 32 * w - pos        # 255 - 32w - pos
            t_b = V + 2 * MS - 1 - 32 * w - pos    # 340 - 32w - pos
            wins[N_VWIN + w, :, 0:L] = (
                fc[:, t_a].transpose(1, 0, 2).reshape(128, L))
            wins[N_VWIN + w, :, L:2 * L] = (
                fc[:, t_b].transpose(1, 0, 2).reshape(128, L))
        xinit = np.zeros((128, 4 * L), dtype=np.float32)
        ii = np.arange(L)
        for b in range(BPC):
            xinit[ii, b * L + ii] = np.exp(fc[b, V + MS - 1] - C_PRE)
            xinit[L + ii, b * L + ii] = np.exp(fc[b, V + 2 * MS - 1] - C_PRE)
        in_maps.append({
            "wins": wins.reshape(NW * 128, 128),
            "xinit": xinit,
            "consts": consts,
            "fgold": np.ascontiguousarray(
                fc.reshape(BPC * T_LEN, L)),
            "oh": np.ascontiguousarray(
                oh[c * BPC:(c + 1) * BPC].reshape(BPC * (T_LEN + 1), L)),
        })
    return in_maps


def kernel(feats, tags, T):
    global _compiled
    from concourse.bass_utils import run_bass_kernel_spmd

    if _compiled is None:
        _compiled = _build_program()
    nc = _compiled

    in_maps = _prep_in_maps(feats, tags, T)
    res = run_bass_kernel_spmd(nc, in_maps, list(range(N_CORES)))
    out = np.concatenate(
        [res.results[c]["out"].reshape(BPC) for c in range(N_CORES)])
    return out.astype(np.float32)


# revision 8
# speedup vs baseline: 1.4028x; 1.1230x over previous
"""CRF negative-log-likelihood loss kernel for Trainium2 (8 NeuronCores).

Strategy
--------
Data-parallel over the batch: 32 sequences -> 4 per core. The log-partition
function is evaluated in linear space with a 4-way split of the time axis
that cuts the serial chain to 85 rounds:

  vec-fwd : alpha_t = e_t * (M @ alpha_{t-1}),  t = 1..170   (M = exp(T))
  matA    : X <- D_s (M^T X), s = 254..171, X0 = diag(e_255)
            => alpha_255 = X_A^T (M alpha_170)   [64x64 transfer matrix]
  matB    : X <- D_s (M^T X), s = 339..256, X0 = diag(e_340)
            => alpha_340 = X_B^T (M alpha_255)
  vec-bwd : v_t = e_t * (M^T v_{t+1}),  t = 510..341, v_511 = e_511
  combine : Z = (M alpha_340) . v_341

Both vector chains live stacked in one [128,4] state: one matmul against a
constant block-diagonal weight blkdiag(exp(T)^T, exp(T)) advances fwd+bwd
together; one DVE multiply applies both emissions. The two matrix chains
live stacked in one [128, 4*64] state with weight blkdiag(exp(T), exp(T));
their per-step row-scale by e_s reads the emission tile through a stride-0
broadcast AP, split across DVE and GpSimd (both can read PSUM) so the DVE
stays under the round budget. Each round advances the vector chains TWO
steps and the matrix chains ONE step (the matrix step's [128,256] psum->sbuf
move is slower, 171/85 balances the two serial cadences).

Emissions are prescaled: e_t = exp(feat_t - C) with C = ln(64)+1 (the mean
per-step growth), which keeps every chain's magnitude within e^+-8 over the
whole sequence - NO renormalization events at all. The final
logZ = Ln(sum g) + 512*C.

The gold-path score (emissions at tags plus transitions) uses one-hot
matmuls identical to the loop's (128,128) PE tiling, placed after the loop.

Host-side work is limited to input relayout: window packing of feats (each
128x128 window pairs two 32-step chunks so one DMA-transpose yields both
chains' emission columns), one-hot tag encoding, and tiny init diagonals.
"""

import math

import numpy as np
from contextlib import ExitStack

B, T_LEN, L = 32, 512, 64
N_CORES = 8
BPC = B // N_CORES   # sequences per core
V = 171              # vector-chain steps per direction (e_0..e_170 fwd)
MS = 85              # matrix-chain steps per direction
C_PRE = math.log(64.0) + 1.0
N_ROUNDS = MS        # 85 rounds; 2 vec steps + 1 mat step per round
N_VWIN = 6           # vec windows (ceil(171/32))
N_MWIN = 3           # mat windows (ceil(85/32))

_compiled = None


def _build_program():
    import concourse.bacc as bacc
    import concourse.tile as tile
    import concourse.mybir as mybir
    from concourse.alu_op_type import AluOpType

    f32 = mybir.dt.float32
    bf16 = mybir.dt.bfloat16
    Af = mybir.ActivationFunctionType

    nc = bacc.Bacc("TRN2", target_bir_lowering=False, debug=False,
                   num_devices=N_CORES)

    NW = N_VWIN + N_MWIN
    wins_d = nc.dram_tensor("wins", [NW * 128, 128], f32,
                            kind="ExternalInput").ap()
    xinit_d = nc.dram_tensor("xinit", [128, 4 * L], f32,
                             kind="ExternalInput").ap()
    consts_d = nc.dram_tensor("consts", [5 * L, L], f32,
                              kind="ExternalInput").ap()
    fgold_d = nc.dram_tensor("fgold", [BPC * T_LEN, L], f32,
                             kind="ExternalInput").ap()
    oh_d = nc.dram_tensor("oh", [BPC * (T_LEN + 1), L], f32,
                          kind="ExternalInput").ap()
    out_d = nc.dram_tensor("out", [1, BPC], f32, kind="ExternalOutput").ap()

    with tile.TileContext(nc) as tc, ExitStack() as ctx:
        consts = ctx.enter_context(tc.tile_pool(name="consts", bufs=1))
        loadp = ctx.enter_context(tc.tile_pool(name="load", bufs=3))
        goldp = ctx.enter_context(tc.tile_pool(name="gold", bufs=32))
        alphap = ctx.enter_context(tc.tile_pool(name="alpha", bufs=6))
        xp = ctx.enter_context(tc.tile_pool(name="xstate", bufs=3))
        vtmp = ctx.enter_context(tc.tile_pool(name="vtmp", bufs=6))
        qv = ctx.enter_context(tc.tile_pool(name="qvpsum", bufs=3,
                                            space="PSUM"))
        qm = ctx.enter_context(tc.tile_pool(name="qmpsum", bufs=3,
                                            space="PSUM"))
        tpp = ctx.enter_context(tc.tile_pool(name="tpsum", bufs=2,
                                             space="PSUM"))

        # ---- constants ----
        ones128 = consts.tile([128, 1], f32)
        nc.gpsimd.memset(ones128[:], 1.0)
        ones128b = consts.tile([128, 1], bf16)
        nc.gpsimd.memset(ones128b[:], 1.0)
        mask_sb = consts.tile([128, L], f32)     # [eye(64); T]
        nc.sync.dma_start(out=mask_sb[:], in_=consts_d[0:2 * L, :])
        tt0 = consts.tile([L, L], f32)           # T^T at partitions 0:64
        nc.sync.dma_start(out=tt0[:], in_=consts_d[2 * L:3 * L, :])
        ts0 = consts.tile([L, L], f32)           # T  at partitions 0:64
        nc.sync.dma_start(out=ts0[:], in_=consts_d[3 * L:4 * L, :])
        ttT = consts.tile([128, L], f32)         # T^T at partitions 64:128
        nc.sync.dma_start(out=ttT[L:128, :], in_=consts_d[4 * L:5 * L, :])
        xi_f = consts.tile([128, 4 * L], f32)
        nc.sync.dma_start(out=xi_f[:], in_=xinit_d)

        # W_vec = blkdiag(exp(T)^T, exp(T)); W_mat = blkdiag(exp(T), exp(T))
        Wv = consts.tile([128, 128], bf16)
        nc.gpsimd.memset(Wv[:], 0.0)
        nc.scalar.activation(Wv[0:L, 0:L], tt0[:], Af.Exp)
        nc.scalar.activation(Wv[L:128, L:128], mask_sb[L:128, :], Af.Exp)
        Wm = consts.tile([128, 128], bf16)
        nc.gpsimd.memset(Wm[:], 0.0)
        nc.scalar.activation(Wm[0:L, 0:L], ts0[:], Af.Exp)
        nc.scalar.activation(Wm[L:128, L:128], mask_sb[L:128, :], Af.Exp)
        # exp(T)^T at partitions 64:128 (tail weight for y3 = M a3)
        Wy3 = consts.tile([128, L], bf16)
        nc.scalar.activation(Wy3[L:128, :], ttT[L:128, :], Af.Exp)

        # ---- emission windows: DMA -> Exp(x - C) bf16 -> DMA-transpose ----
        # Window tile ek[k]: [128 part = (chainTop L | chainBot L),
        #                     128 free = 4*pos + b].
        negC = consts.tile([128, 1], f32)
        nc.gpsimd.memset(negC[:], -C_PRE)
        eks = [None] * NW

        def load_window(k):
            fcg = loadp.tile([128, 128], f32, tag=f"fc{k}")
            nc.sync.dma_start(out=fcg[:],
                              in_=wins_d[k * 128:(k + 1) * 128, :])
            stg = loadp.tile([128, 128], bf16, tag=f"stg{k}")
            nc.scalar.activation(stg[:], fcg[:], Af.Exp, bias=negC[:])
            return stg

        def transpose_window(k, stg):
            ek = consts.tile([128, 128], bf16, tag=f"ef{k}")
            nc.sync.dma_start(out=ek[:], in_=stg[:], transpose=True)
            eks[k] = ek

        # first vec + first mat window end-to-end, then the rest
        stg0 = load_window(0)
        stgm = load_window(N_VWIN)
        transpose_window(0, stg0)
        transpose_window(N_VWIN, stgm)
        rest = [k for k in range(NW) if k not in (0, N_VWIN)]
        stgs = {k: load_window(k) for k in rest}
        for k in rest:
            transpose_window(k, stgs[k])

        def e_vec(idx):   # [128,4] emissions for vec round-step idx
            w, p = idx // 32, idx % 32
            return eks[w][:, 4 * p:4 * p + 4]

        def e_mat(idx):   # [128,4] emissions for mat step idx
            w, p = idx // 32, idx % 32
            return eks[N_VWIN + w][:, 4 * p:4 * p + 4]

        # ---- init states ----
        S = alphap.tile([128, BPC], bf16, tag="S")
        nc.vector.tensor_copy(S[:], e_vec(0))      # [alpha_0 ; v_511]
        X = xp.tile([128, 4 * L], bf16, tag="X")
        nc.vector.tensor_copy(X[:], xi_f[:])       # [diag e_255 ; diag e_340]

        # ---- main loop: 2 vec steps + 1 mat step per round ----
        def vec_step(idx):
            nonlocal S
            q = qv.tile([128, BPC], f32, tag="q")
            nc.tensor.matmul(q[:], lhsT=Wv[:], rhs=S[:],
                             start=True, stop=True)
            S_new = alphap.tile([128, BPC], bf16, tag="S")
            nc.vector.tensor_mul(S_new[:], q[:], e_vec(idx))
            S = S_new

        for r in range(N_ROUNDS):
            vec_step(1 + 2 * r)
            if r < MS - 1:
                p = qm.tile([128, 4 * L], f32, tag="p")
                nc.tensor.matmul(p[:], lhsT=Wm[:], rhs=X[:],
                                 start=True, stop=True)
                em = e_mat(1 + r)
                X_new = xp.tile([128, 4 * L], bf16, tag="X")
                # row-scale by e: free dim (b, i) with e broadcast over i
                pb = p[:].rearrange("a (b i) -> a b i", b=4)
                xb = X_new[:].rearrange("a (b i) -> a b i", b=4)
                nc.vector.tensor_mul(
                    xb[:], pb[:],
                    em[:].unsqueeze(2).broadcast_to((128, 4, L)))
                X = X_new
            vec_step(2 + 2 * r)

        # ---- tail: stitch the four chains together ----
        # y1 = M alpha_170 (rows 0:64 of a W_vec matmul)
        qy1 = qv.tile([128, BPC], f32, tag="q")
        nc.tensor.matmul(qy1[:], lhsT=Wv[:], rhs=S[:], start=True, stop=True)
        y1 = vtmp.tile([L, BPC], bf16, tag="y1")
        nc.vector.tensor_copy(y1[:], qy1[0:L, :])
        # a2 = X_A^T y1  (per-sequence 64x64 blocks, rows 0:64 of X)
        qa2 = qv.tile([128, BPC], f32, tag="q")
        for b in range(BPC):
            nc.tensor.matmul(qa2[0:L, b:b + 1],
                             lhsT=X[0:L, b * L:(b + 1) * L],
                             rhs=y1[:, b:b + 1], start=True, stop=True)
        a2 = vtmp.tile([L, BPC], bf16, tag="a2")
        nc.vector.tensor_copy(a2[:], qa2[0:L, :])
        # y2 = M a2 -> psum partitions 64:128 (to meet X_B's partition base)
        qy2 = qv.tile([128, BPC], f32, tag="q")
        nc.tensor.matmul(qy2[L:128, :], lhsT=Wv[0:L, 0:L], rhs=a2[:],
                         start=True, stop=True, tile_position=(0, 64))
        y2 = vtmp.tile([128, BPC], bf16, tag="y2")
        nc.vector.tensor_copy(y2[L:128, :], qy2[L:128, :])
        # a3 = X_B^T y2  (rows 64:128 of X)
        qa3 = qv.tile([128, BPC], f32, tag="q")
        for b in range(BPC):
            nc.tensor.matmul(qa3[L:128, b:b + 1],
                             lhsT=X[L:128, b * L:(b + 1) * L],
                             rhs=y2[L:128, b:b + 1], start=True, stop=True,
                             tile_position=(64, 64))
        a3 = vtmp.tile([128, BPC], bf16, tag="a3")
        nc.vector.tensor_copy(a3[L:128, :], qa3[L:128, :])
        # y3 = M a3; g = y3 * v_341; Z = sum(g)
        qy3 = qv.tile([128, BPC], f32, tag="q")
        nc.tensor.matmul(qy3[L:128, :], lhsT=Wy3[L:128, :], rhs=a3[L:128, :],
                         start=True, stop=True, tile_position=(64, 64))
        g = vtmp.tile([128, BPC], bf16, tag="g")
        nc.vector.tensor_mul(g[L:128, :], qy3[L:128, :], S[L:128, :])
        qz = qv.tile([128, BPC], f32, tag="q")
        qz_inst = nc.tensor.matmul(qz[0:1, :], lhsT=ones128b[L:128, 0:1],
                                   rhs=g[L:128, :], start=True, stop=True,
                                   tile_position=(64, 0))
        lnZ = vtmp.tile([1, BPC], f32, tag="lnZ")
        nc.scalar.activation(lnZ[:], qz[0:1, :], Af.Ln)
        lnoff = consts.tile([1, BPC], f32)
        nc.gpsimd.memset(lnoff[:], float(T_LEN * C_PRE))
        fwd2 = vtmp.tile([1, BPC], f32, tag="fwd2")
        nc.vector.tensor_add(fwd2[:], lnZ[:], lnoff[:])

        # ---- gold score via one-hot matmuls, forced after the loop ----
        # bf16 operands (one-hot is exact; feats rounding is ~1e-3 abs) so
        # the PE never switches into the slow two-pass fp32 weight mode.
        from concourse.tile_rust import add_dep_helper
        Vt = consts.tile([128, BPC], f32)
        for b in range(BPC):
            gps = tpp.tile([128, L], f32, tag="tp")
            for c in range(4):
                o0 = b * (T_LEN + 1) + c * 128
                catf = goldp.tile([128, 128], f32, tag="catf")
                nc.sync.dma_start(
                    out=catf[:, 0:L],
                    in_=fgold_d[b * T_LEN + c * 128:b * T_LEN + (c + 1) * 128,
                                :])
                nc.sync.dma_start(out=catf[:, L:2 * L],
                                  in_=oh_d[o0 + 1:o0 + 129, :])
                ohpf = goldp.tile([128, L], f32, tag="ohpf")
                nc.sync.dma_start(out=ohpf[:], in_=oh_d[o0:o0 + 128, :])
                cat = goldp.tile([128, 128], bf16, tag="cat")
                nc.gpsimd.tensor_copy(cat[:], catf[:])
                ohp = goldp.tile([128, L], bf16, tag="ohp")
                nc.gpsimd.tensor_copy(ohp[:], ohpf[:])
                gi = nc.tensor.matmul(gps[:], lhsT=cat[:], rhs=ohp[:],
                                      start=(c == 0), stop=(c == 3))
                add_dep_helper(gi.ins, qz_inst.ins, sync=True,
                               reason="gold matmuls after recurrence")
            gsc = vtmp.tile([128, L], f32, tag="gsc")
            nc.vector.tensor_mul(gsc[:], gps[:], mask_sb[:])
            nc.vector.tensor_reduce(Vt[:, b:b + 1], gsc[:],
                                    axis=mybir.AxisListType.X,
                                    op=AluOpType.add)
        gold_ps = tpp.tile([128, L], f32, tag="tp")
        nc.tensor.matmul(gold_ps[0:1, 0:BPC], lhsT=ones128[:, 0:1], rhs=Vt[:],
                         start=True, stop=True)

        res = vtmp.tile([1, BPC], f32, tag="res")
        nc.vector.tensor_tensor(res[:], fwd2[:], gold_ps[0:1, 0:BPC],
                                op=AluOpType.subtract)
        nc.sync.dma_start(out=out_d, in_=res[:])

    import concourse.bacc as bacc2
    orig = bacc2.Bacc.move_matmul_waits_to_ldweights
    if SKIP_LDW_WAIT_PASS:
        # Keep semaphore waits on the MATMUL itself so the LDWEIGHTS can
        # issue while the previous round's DVE multiply runs.
        bacc2.Bacc.move_matmul_waits_to_ldweights = lambda self: None
    try:
        nc.compile()
    finally:
        bacc2.Bacc.move_matmul_waits_to_ldweights = orig
    return nc


SKIP_LDW_WAIT_PASS = True


def _prep_in_maps(feats, tags, T):
    feats = np.ascontiguousarray(np.asarray(feats, dtype=np.float32))
    T_np = np.ascontiguousarray(np.asarray(T, dtype=np.float32))
    tags_np = np.asarray(tags).astype(np.int64)

    oh = np.zeros((B, T_LEN + 1, L), dtype=np.float32)
    oh[np.arange(B)[:, None], np.arange(T_LEN)[None, :], tags_np] = 1.0
    consts = np.concatenate(
        [np.eye(L, dtype=np.float32), T_np, T_np.T, T_np, T_np.T], axis=0)
    consts = np.ascontiguousarray(consts)

    pos = np.arange(32)
    NW = N_VWIN + N_MWIN
    in_maps = []
    for c in range(N_CORES):
        fc = feats[c * BPC:(c + 1) * BPC]          # [4, 512, 64]
        wins = np.zeros((NW, 128, 128), dtype=np.float32)
        for w in range(N_VWIN):
            t_top = np.minimum(32 * w + pos, T_LEN - 1)
            t_bot = T_LEN - 1 - 32 * w - pos
            # [4, 32, 64] -> [32, 4, 64] -> [128, 64]
            wins[w, :, 0:L] = fc[:, t_top].transpose(1, 0, 2).reshape(128, L)
            wins[w, :, L:2 * L] = (
                fc[:, t_bot].transpose(1, 0, 2).reshape(128, L))
        for w in range(N_MWIN):
            t_a = V + MS - 1 - 32 * w - pos        # 255 - 32w - pos
            t_b = V + 2 * MS - 1 - 32 * w - pos    # 340 - 32w - pos
            wins[N_VWIN + w, :, 0:L] = (
                fc[:, t_a].transpose(1, 0, 2).reshape(128, L))
            wins[N_VWIN + w, :, L:2 * L] = (
                fc[:, t_b].transpose(1, 0, 2).reshape(128, L))
        xinit = np.zeros((128, 4 * L), dtype=np.float32)
        ii = np.arange(L)
        for b in range(BPC):
            xinit[ii, b * L + ii] = np.exp(fc[b, V + MS - 1] - C_PRE)
            xinit[L + ii, b * L + ii] = np.exp(fc[b, V + 2 * MS - 1] - C_PRE)
        in_maps.append({
            "wins": wins.reshape(NW * 128, 128),
            "xinit": xinit,
            "consts": consts,
            "fgold": np.ascontiguousarray(
                fc.reshape(BPC * T_LEN, L)),
            "oh": np.ascontiguousarray(
                oh[c * BPC:(c + 1) * BPC].reshape(BPC * (T_LEN + 1), L)),
        })
    return in_maps


def kernel(feats, tags, T):
    global _compiled
    from concourse.bass_utils import run_bass_kernel_spmd

    if _compiled is None:
        _compiled = _build_program()
    nc = _compiled

    in_maps = _prep_in_maps(feats, tags, T)
    res = run_bass_kernel_spmd(nc, in_maps, list(range(N_CORES)))
    out = np.concatenate(
        [res.results[c]["out"].reshape(BPC) for c in range(N_CORES)])
    return out.astype(np.float32)
